# revision 5
# baseline (speedup 1.0000x reference)
"""Expected Calibration Error (histogram binning) on 8 Trainium2 NeuronCores.

fp16 streaming kernel (~96.5 us cost model vs 169 us f32 predecessor).

Host prep: per row, subtract the true-class logit from every logit (all
ECE statistics survive this shift: p = 1/(1+sum exp(d)), correct iff
max(d) <= 0 with sign preserved exactly by f32 subtraction), drop the
now-constant target column, clip at +11 so exp stays inside fp16, cast
fp16. 99 fp16 columns = half the f32 HBM bytes; the cost-model DMA floor
drops to ~69 us/core at 360 GB/s. Rows are packed [8 cores, 128
partitions, 980 rows]; pad rows use d=+11 (p = 1/(1+99*e^11) > 0, never
correct) and are removed by an exact count correction on C_0.

Device, streaming chunks of geometrically ramped width (keeps ACT
gapless from the first DMA; ~67 rows/partition steady state):
  ACT : exp of the whole chunk, fp16 in/out - the critical engine
        (84 us busy; no 16-bit speedup exists on ACT)
  DVE : row max via packed-fp16 tensor_tensor folds 99->33->11 + a
        short tensor_reduce (the folds run at the 2x 16-bit rate;
        tensor_reduce alone would not)
  PE  : row sum of exp via accumulating fp16 identity matmuls straight
        into a per-slab PSUM strip, +1 via a ones-column matmul
Finish per slab of chunks: p = reciprocal(PSUM) written as fp16,
corr = (M <= 0), z = p*corr (Pool mult - the only ALU op GPSIMD
implements besides add), then per boundary b, accumulator-fused DVE
scans (Pool cannot run accum scans on silicon; with accum_out, op1 is
the REDUCTION op, so the relu sum takes two passes):
  C_b = count(p > b)      b=0..0.9   (is_gt, add-accum)
  R_b = sum relu(p - b)   b=0..0.9   (sub+max pass, bypass+add-accum)
  Z_b = count(z > b)      b=0..0.4   (p > 0.5 implies correct, so
                                      Z_b = C_b for b >= 0.5)
A few final-slab R scans run on the then-idle ACT (fused relu+accum),
and all but the last slab's accumulators are DMA'd out early.
Host: f64-sum the 8x128 partials, SP_b = R_b + b*C_b, finish the ECE
scalar exactly as the reference does.
"""

import os
import sys
import tempfile

import numpy as np

if "/opt/trn_rl_repo" not in sys.path:
    sys.path.insert(0, "/opt/trn_rl_repo")

os.environ.setdefault(
    "JAX_COMPILATION_CACHE_DIR",
    os.path.join(tempfile.gettempdir(), "jaxcache"),
)

N = 1_000_000
C = 100
C99 = C - 1
NCORES = 8
P = 128
# rows per partition; 977 = ceil(1e6 / (8*128)) is the minimum (980 would
# stream 0.31% dead pad work through every engine)
W = int(os.environ.get("KV_W", "977"))
assert NCORES * P * W >= N
PECLS = int(os.environ.get("KV_PECLS", "99"))


def _parse_widths(s):
    out = []
    for part in s.split(","):
        if "x" in part:
            n, w = part.split("x")
            out.extend([int(w)] * int(n))
        else:
            out.append(int(part))
    return out


# row-group widths per chunk: geometric ramp keeps ACT gapless from the first
# DMA, small tail chunks shorten the drain; tuned against TimelineSim
_WIDTHS = _parse_widths(
    os.environ.get("KV_WIDTHS", "8,9,13,18,24,29,36,45,56,68,78,8x67,33,16,8")
)
assert sum(_WIDTHS) == W, _WIDTHS
NCHUNK = len(_WIDTHS)
_STARTS = [0]
for w_ in _WIDTHS:
    _STARTS.append(_STARTS[-1] + w_)
# slab ends as chunk indices
_SLAB_ENDS = [
    int(v) for v in
    os.environ.get("KV_SLABS", "7,11,15,18,22").split(",")
]
_FOLD34 = os.environ.get("KV_FOLD34", "dve")
_TAILACT = int(os.environ.get("KV_TAILACT", "3"))
_EXP_SPLITS = [int(v) for v in os.environ.get("KV_EXPSPLIT", "").split(",") if v]
_LASTSPLIT = int(os.environ.get("KV_LASTSPLIT", "50"))
# 1: test positivity of cols 66:99 via is_gt mask (DVE 4x rate) + PE count
# instead of folding them into the max tree (DVE 2x rate)
_PECNT = int(os.environ.get("KV_PECNT", "0"))
NSLAB = len(_SLAB_ENDS)
XBUFS = int(os.environ.get("KV_XBUFS", "5"))
EBUFS = int(os.environ.get("KV_EBUFS", "5"))
JBUFS = int(os.environ.get("KV_JBUFS", "8"))
NPAD = NCORES * P * W
PADVAL = 11.0
CLIP = 11.0
NACC = 25  # 10 C + 10 R + 5 Z per slab

_BOUNDS = np.linspace(0.0, 1.0, 11).astype(np.float32)

_built = {}


def _build_program():
    if "nc" in _built:
        return _built["nc"]

    import concourse.bacc as bacc
    import concourse.tile as tile
    from concourse import mybir

    f32 = mybir.dt.float32
    f16 = mybir.dt.float16
    Alu = mybir.AluOpType
    Act = mybir.ActivationFunctionType
    AxX = mybir.AxisListType.X

    nc = bacc.Bacc("TRN2", target_bir_lowering=False, debug=False)
    x_d = nc.dram_tensor("x", [P, W * C99], f16, kind="ExternalInput").ap()
    ident_d = nc.dram_tensor("ident", [P, P], f16, kind="ExternalInput").ap()
    nbnd_d = nc.dram_tensor("nbnd", [P, 11], f32, kind="ExternalInput").ap()
    acc_d = nc.dram_tensor("acc", [P, NACC * NSLAB], f32, kind="ExternalOutput").ap()

    slab_cols = [0] + [_STARTS[e] for e in _SLAB_ENDS]
    assert slab_cols[-1] == W
    MAXSLAB = max(b - a for a, b in zip(slab_cols, slab_cols[1:]))

    with tile.TileContext(nc) as tc:
        with (
            tc.tile_pool(name="consts", bufs=1) as consts,
            tc.tile_pool(name="stats", bufs=1) as stats,
            tc.tile_pool(name="xin", bufs=XBUFS) as xin,
            tc.tile_pool(name="etmp", bufs=EBUFS) as etmp,
            tc.tile_pool(name="fold", bufs=2) as fold,
            tc.tile_pool(name="junk", bufs=JBUFS) as junkp,
            tc.tile_pool(name="psum", bufs=2, space="PSUM") as psp,
        ):
            ident_t = consts.tile([P, P], f16)
            nc.gpsimd.dma_start(ident_t[:], ident_d[:, :])
            nbnd_t = consts.tile([P, 11], f32)
            nc.gpsimd.dma_start(nbnd_t[:], nbnd_d[:, :])
            ones_t = consts.tile([P, max(_WIDTHS)], f16)
            nc.gpsimd.memset(ones_t[:], 1.0)

            M = stats.tile([P, W], f16, tag="M")
            PT = stats.tile([P, W], f16, tag="PT")
            CORR = stats.tile([P, W], f16, tag="CORR")
            Z = stats.tile([P, W], f16, tag="Z")
            ACC = stats.tile([P, NACC * NSLAB], f32, tag="ACC")

            def finish_slab(si, PS, CNT=None):
                # Accum-fused scans are DVE-only on real TRN2 silicon (the
                # Pool engine ALU only implements add/mult, no accumulators);
                # DVE runs them at the packed-fp16 fast rate. With accum_out,
                # op1 is the REDUCTION op, so relu+sum takes an elementwise
                # relu pass followed by a bypass+add-accum pass.
                c0, c1 = slab_cols[si], slab_cols[si + 1]
                nw = c1 - c0
                with nc.allow_low_precision(reason="p feeds 10-bin histogram"):
                    nc.vector.reciprocal(PT[:, c0:c1], PS[:, :nw])
                nc.vector.tensor_scalar(
                    CORR[:, c0:c1], M[:, c0:c1], 0.0, None, op0=Alu.is_le
                )
                nc.gpsimd.tensor_tensor(
                    Z[:, c0:c1], PT[:, c0:c1], CORR[:, c0:c1], op=Alu.mult
                )
                if CNT is not None:
                    CORR2 = junkp.tile([P, MAXSLAB], f16, name="junk", tag="junk")
                    nc.vector.tensor_scalar(
                        CORR2[:, :nw], CNT[:, :nw], 0.5, None, op0=Alu.is_lt,
                    )
                    nc.gpsimd.tensor_tensor(
                        Z[:, c0:c1], Z[:, c0:c1], CORR2[:, :nw], op=Alu.mult
                    )
                ab = NACC * si

                def jt():
                    j = junkp.tile([P, MAXSLAB], f16, name="junk", tag="junk")
                    return j[:, :nw]

                last = si == NSLAB - 1
                for b in range(10):
                    lo = float(_BOUNDS[b])
                    nc.vector.tensor_scalar(
                        jt(), PT[:, c0:c1], lo, None,
                        op0=Alu.is_gt, op1=Alu.add,
                        accum_out=ACC[:, ab + b:ab + b + 1],
                    )
                    if last and b < _TAILACT:
                        # stream is done - ACT sits idle, its fused relu+sum
                        # (baseline-proven) absorbs a few R scans in parallel
                        nc.scalar.activation(
                            jt(), PT[:, c0:c1], Act.Relu,
                            bias=nbnd_t[:, b:b + 1],
                            accum_out=ACC[:, ab + 10 + b:ab + 11 + b],
                        )
                        continue
                    relu_j = junkp.tile([P, MAXSLAB], f16, name="junk", tag="junk")
                    nc.vector.tensor_scalar(
                        relu_j[:, :nw], PT[:, c0:c1], lo, 0.0,
                        op0=Alu.subtract, op1=Alu.max,
                    )
                    nc.vector.tensor_scalar(
                        jt(), relu_j[:, :nw], 0.0, None,
                        op0=Alu.bypass, op1=Alu.add,
                        accum_out=ACC[:, ab + 10 + b:ab + 11 + b],
                    )
                for b in range(5):
                    lo = float(_BOUNDS[b])
                    nc.vector.tensor_scalar(
                        jt(), Z[:, c0:c1], lo, None,
                        op0=Alu.is_gt, op1=Alu.add,
                        accum_out=ACC[:, ab + 20 + b:ab + 21 + b],
                    )

            _FINDMA = {"sp": nc.sync, "dve": nc.vector, "act": nc.scalar}[
                os.environ.get("KV_FINDMA", "sp")
            ]
            cur_ps = None
            cur_cnt = None
            for k in range(NCHUNK):
                g0, gw = _STARTS[k], _WIDTHS[k]
                si = next(i for i, e in enumerate(_SLAB_ENDS) if k < e)
                # mode 2: count-based corr only for early slabs; the last
                # slab keeps the pure-fold path so its post-stream PE chain
                # stays short
                use_cnt = _PECNT == 1 or (_PECNT == 2 and si < NSLAB - 1)
                if cur_ps is None:
                    cur_ps = psp.tile([P, MAXSLAB], f32, name="PS", tag="PS")
                    if use_cnt:
                        cur_cnt = psp.tile(
                            [P, MAXSLAB], f32, name="CNT", tag="CNT"
                        )
                off = g0 - slab_cols[si]

                X = xin.tile([P, gw * C99], f16)
                dma_eng = (
                    nc.gpsimd
                    if (k == 0 and os.environ.get("KV_DMA0") == "pool")
                    else nc.sync
                )
                dma_eng.dma_start(
                    X[:], x_d[:, g0 * C99:(g0 + gw) * C99]
                )
                x3 = X[:].rearrange("p (g c) -> p g c", c=C99)

                # --- max: packed tensor_tensor folds 99 -> 33 -> 11 + reduce,
                # all DVE (the Pool ALU only implements add/mult on silicon)
                F1 = fold.tile([P, gw * 33], f16)
                f1 = F1[:].rearrange("p (g c) -> p g c", c=33)
                B = fold.tile([P, gw * 11], f16)
                b3 = B[:].rearrange("p (g c) -> p g c", c=11)
                nc.vector.tensor_tensor(
                    f1, x3[:, :, 0:33], x3[:, :, 33:66], op=Alu.max
                )
                if use_cnt:
                    G2 = fold.tile([P, gw * 33], f16)
                    g23 = G2[:].rearrange("p (g c) -> p g c", c=33)
                    nc.vector.tensor_scalar(
                        g23, x3[:, :, 66:99], 0.0, None, op0=Alu.is_gt
                    )
                else:
                    nc.vector.tensor_tensor(
                        f1, f1, x3[:, :, 66:99], op=Alu.max
                    )
                nc.vector.tensor_tensor(
                    b3, f1[:, :, 0:11], f1[:, :, 11:22], op=Alu.max
                )
                nc.vector.tensor_tensor(b3, b3, f1[:, :, 22:33], op=Alu.max)
                nc.vector.tensor_reduce(
                    M[:, g0:g0 + gw], b3, axis=AxX, op=Alu.max
                )
                if use_cnt:
                    for cc in range(33):
                        nc.tensor.matmul(
                            cur_cnt[:, off:off + gw], ident_t[:],
                            g23[:, :, cc:cc + 1].rearrange("p g c -> p (g c)"),
                            start=(cc == 0), stop=(cc == 32),
                        )

                # --- exp (fp16 out; host clip keeps it finite); last chunk is
                # split so PE overlaps the second half at the pipeline tail
                E = etmp.tile([P, gw * C99], f16)
                e3 = E[:].rearrange("p (g c) -> p g c", c=C99)
                esplit = [0] + _EXP_SPLITS + [C99]
                if k == NCHUNK - 1 and _LASTSPLIT:
                    esplit = [0, _LASTSPLIT, C99]
                for a, b in zip(esplit, esplit[1:]):
                    nc.scalar.activation(e3[:, :, a:b], x3[:, :, a:b], Act.Exp)

                # --- row sums: PE identity matmuls straight into the slab's
                # PSUM strip (+1 via ones col); reciprocal later reads PSUM
                nc.tensor.matmul(
                    cur_ps[:, off:off + gw], ident_t[:], ones_t[:, 0:gw],
                    start=True, stop=False,
                )
                for cc in range(C99):
                    nc.tensor.matmul(
                        cur_ps[:, off:off + gw], ident_t[:],
                        e3[:, :, cc:cc + 1].rearrange("p g c -> p (g c)"),
                        start=False, stop=(cc == C99 - 1),
                    )

                if (k + 1) in _SLAB_ENDS:
                    finish_slab(si, cur_ps, cur_cnt)
                    cur_ps = None
                    cur_cnt = None
                    if si == NSLAB - 2:
                        # ship all but the last slab's accumulators early so
                        # only 25 columns gate the final DMA
                        nc.sync.dma_start(
                            acc_d[:, :NACC * (NSLAB - 1)],
                            ACC[:, :NACC * (NSLAB - 1)],
                        )

            # final 25 accumulator columns: issue from the engine that wrote
            # them last so the DMA doesn't pay a cross-engine sem hop
            _FINDMA.dma_start(
                acc_d[:, NACC * (NSLAB - 1):], ACC[:, NACC * (NSLAB - 1):]
            )

    nc.compile()
    _built["nc"] = nc
    return nc


def _prep_inputs(outputs, targets):
    """Sort rows by class, roll so the target col leads, subtract it, drop it,
    clip at +11, cast fp16, pack densely onto the [8, 128, 980] grid."""
    x = np.ascontiguousarray(np.asarray(outputs, dtype=np.float32))
    t = np.asarray(targets).astype(np.int64).ravel()
    order = np.argsort(t, kind="stable")
    cnt = np.bincount(t, minlength=C)
    starts = np.zeros(C + 1, np.int64)
    starts[1:] = np.cumsum(cnt)

    Xr = np.empty((NPAD, C99), np.float16)
    for c in range(C):
        s0, s1 = starts[c], starts[c + 1]
        if s1 == s0:
            continue
        src = x[order[s0:s1]]
        xt = src[:, c:c + 1]
        left = np.clip(src[:, c + 1:] - xt, None, CLIP)
        Xr[s0:s1, :C - c - 1] = left.astype(np.float16)
        if c:
            right = np.clip(src[:, :c] - xt, None, CLIP)
            Xr[s0:s1, C - c - 1:] = right.astype(np.float16)
    Xr[N:] = PADVAL

    Xv = Xr.reshape(NCORES, P, W * C99)
    ident = np.eye(P, dtype=np.float16)
    nbnd = np.broadcast_to(-_BOUNDS, (P, 11)).astype(np.float32).copy()
    return [{"x": Xv[c], "ident": ident, "nbnd": nbnd} for c in range(NCORES)]


def _postprocess(acc_list):
    A = np.stack(acc_list)
    tot = A.astype(np.float64).sum(axis=(0, 1))
    tot = tot.reshape(NSLAB, NACC).sum(axis=0)
    bounds = _BOUNDS.astype(np.float64)
    Cg = np.zeros(11)
    R = np.zeros(11)
    Zg = np.zeros(11)
    Cg[:10] = tot[0:10]
    R[:10] = tot[10:20]
    Zg[:5] = tot[20:25]
    Zg[5:10] = Cg[5:10]          # p > 0.5 implies correct
    Cg[0] -= NPAD - N            # pad rows (p = 1/(1+99*e^11) > 0) land in bin 0
    SP = R + bounds * Cg
    cnt = Cg[:10] - Cg[1:]
    sp = SP[:10] - SP[1:]
    sc = Zg[:10] - Zg[1:]
    nonempty = cnt > 0
    ece = np.sum(np.where(nonempty, np.abs(sp - sc), 0.0))
    total = cnt.sum()
    val = ece / max(total, 1.0) if total > 0 else 0.0
    return np.float32(val)


def _exec(in_maps, trace=False):
    from concourse.bass_utils import run_bass_kernel_spmd

    nc = _build_program()
    res = run_bass_kernel_spmd(
        nc, in_maps, core_ids=list(range(NCORES)), trace=trace
    )
    return [res.results[c]["acc"] for c in range(NCORES)], res


def _subrun(tmpdir):
    in_maps = []
    for c in range(NCORES):
        in_maps.append({
            "x": np.load(f"{tmpdir}/x{c}.npy"),
            "ident": np.load(f"{tmpdir}/ident.npy"),
            "nbnd": np.load(f"{tmpdir}/nbnd.npy"),
        })
    accs, _ = _exec(in_maps)
    np.save(f"{tmpdir}/accs.npy", np.stack(accs))


def _exec_subprocess(in_maps):
    """Fresh-process retry for transient device-unrecoverable errors."""
    import subprocess

    here = os.path.dirname(os.path.abspath(__file__))
    me = os.path.splitext(os.path.basename(__file__))[0]
    with tempfile.TemporaryDirectory() as td:
        for c in range(NCORES):
            np.save(f"{td}/x{c}.npy", in_maps[c]["x"])
        np.save(f"{td}/ident.npy", in_maps[0]["ident"])
        np.save(f"{td}/nbnd.npy", in_maps[0]["nbnd"])
        code = (
            f"import sys; sys.path.insert(0, {here!r}); "
            f"import {me} as K; K._subrun({td!r})"
        )
        subprocess.run([sys.executable, "-c", code], check=True, timeout=2400)
        accs = np.load(f"{td}/accs.npy")
    return [accs[c] for c in range(NCORES)]


def _run(outputs, targets, trace=False):
    import time

    in_maps = _prep_inputs(outputs, targets)
    accs = None
    last_err = None
    try:
        accs, res = _exec(in_maps, trace=trace)
    except Exception as e:
        last_err = e
        res = None
        sys.stderr.write(f"kernel: in-process exec failed: {e}\n")
    if accs is None:
        for attempt in range(3):
            try:
                time.sleep(5.0)
                accs = _exec_subprocess(in_maps)
                break
            except Exception as e:
                last_err = e
                sys.stderr.write(
                    f"kernel: subprocess exec attempt {attempt} failed: {e}\n"
                )
        else:
            raise last_err
    val = _postprocess(accs)
    return val, res


def kernel(outputs, targets):
    val, _ = _run(outputs, targets, trace=False)
    return val
